# revision 35
# baseline (speedup 1.0000x reference)
"""Trainium2 Bass kernel for nn_CausalWanSelfAttention_45904610460041.

Strategy (8 NeuronCores, full I/O):
  Launch 1 (pair-split, transposed projections): cores 2p/2p+1 share 390
    rows; core 2p computes wq + wv[:, :768], core 2p+1 wk + wv[:, 768:]
    (column-complete, so RMS stays core-local; per-core weight DMA
    ~7.1MB).  The GEMM runs transposed (out^T = W^T x^T): M = 18 exact
    128-col chunks (zero M-padding), N = 390 rows in a single PSUM bank.
    RoPE's pair-swap is a PE permutation matmul; RMS sum-of-squares is a
    bf16 square + fp32 pairwise tree + ones-matmul partition reduce +
    Sqrt + fast reciprocal + ones-broadcast matmul, applied fused with
    the output cast.  T chunks run first so the normalize tail hides
    under the V chunks' PE stream.
  Host glue: applies the (numpy, index-only) KV-cache roll/update/window
    logic and assembles the effective 4680-key K^T / V tensors per head
    (cache rows pass through; softmax is permutation-invariant so key
    order is free).  Partial out-projection sums + bias also on host.
  Launch 2 (2D head x query sharded): core (hg, qh) takes 3 heads x 780
    queries x all 4680 keys (per-core KV DMA 7.2MB).  Query blocks of
    512/268, logits^T = K^T-chunk . Q (keys on partitions), one exp per
    3-chunk PSUM group on ScalarE (the hard floor: ~86k lane-elems),
    P.V accumulated per block.  The last two PV groups of each head are
    held back and drained at the next head's start, filling the
    exp-latency pipeline bubble.  Softmax denominators via a bf16
    pairwise tree emitted PIECEWISE while groups complete (acc1 during
    g6-g11, levels at g11, final add + gpsimd partition_all_reduce at
    g12), so each head's chain is short: PSUM-stage copy + fast
    reciprocal + multiply.  The per-head-group partial out-projection is
    interleaved into the second query block's stream (the scheduler
    hoists rounds right behind the o3 normalizes); only the last block's
    9 rounds trail, double-buffered on the then-idle QK PSUM banks.

  Measured on trn2 (8 cores, fast clock state): launch1 ~70us + launch2
  ~151us = ~224us total HW exec (run-to-run chip clock variance ~15%);
  relative error vs the fp32 reference ~7.1e-3 (matmul operands bf16,
  fp32 accumulation).
"""

import os
import sys

for _p in ("/opt/trn_rl_repo",):
    if os.path.isdir(_p) and _p not in sys.path:
        sys.path.insert(0, _p)

import numpy as np
import ml_dtypes

import concourse.bass as bass
import concourse.tile as tile
from concourse import bacc
from concourse import mybir
from concourse import bass_utils
from concourse import bass_isa
from concourse.alu_op_type import AluOpType

BF16 = ml_dtypes.bfloat16
AF = mybir.ActivationFunctionType

# ---------------------------------------------------------------------------
# Problem constants (fixed by the input specs).
S = 1560          # query/new-token sequence length
DIM = 1536
NH = 12
HD = 128
CACHE = 4680      # kv cache length == effective attention keys here
NCORES = 8
RPC = S // NCORES  # 195 rows (queries) per core
EPS = 1e-6
LOCAL_ATTN_SIZE = 3
SINK_SIZE = 1
MAX_ATTN = 32760 if LOCAL_ATTN_SIZE == -1 else LOCAL_ATTN_SIZE * S

NKC = (CACHE + 127) // 128      # 37 key chunks
TAIL = CACHE - (NKC - 1) * 128  # 72 keys in the tail chunk

# Row-chunk split of the 195 per-core rows into <=128-partition chunks.
RCHUNKS = [(0, 128), (128, 195)]

_CACHED = {}
LAST_RUNS = []  # BassKernelResults of the most recent kernel() call


# ---------------------------------------------------------------------------
# Launch 1 (pair-split, transposed): cores 2p/2p+1 share rows
# [390p, 390p+390).  Core 2p computes all of wq plus wv[:, :768]; core
# 2p+1 all of wk plus wv[:, 768:] (column-complete -> RMS stays local).
# The matmul is TRANSPOSED (out^T = W^T x^T): M = output columns (18
# exact 128-chunks, zero M-padding), N = 390 rows (single PSUM bank),
# so PE streams 18*12*390 columns instead of the row-major 4*12*2304.
# RoPE's pair-swap is a PE permutation matmul; the RMS sum-of-squares
# partition reduction is a ones-vector matmul; rsqrt is Sqrt+fast-recip;
# the per-row 1/rms scale is broadcast back with a second tiny matmul
# and applied fused with the output cast.  T (q/k) chunks run first so
# the normalize tail overlaps the V chunks' PE stream.  The attention
# scale and g vectors are folded into the host-built tables (g must be
# identical across heads -- asserted; it is all-ones here).
RPB = 390                 # rows per core (pair rows)
NMCH = 18                 # output column chunks (12 T + 6 V)


def _build_launch1():
    nc = bacc.Bacc("TRN2", target_bir_lowering=False, debug=False,
                   num_devices=NCORES, num_swdge_queues=4)
    f32, bf = mybir.dt.float32, mybir.dt.bfloat16

    xt_d = nc.dram_tensor("xt", [128, 12, RPB], bf, kind="ExternalInput")
    wt_d = nc.dram_tensor("wt", [NMCH, 128, 12, 128], bf,
                          kind="ExternalInput")
    ct_d = nc.dram_tensor("ct", [128, RPB], bf, kind="ExternalInput")
    st_d = nc.dram_tensor("st", [128, RPB], bf, kind="ExternalInput")
    perm_d = nc.dram_tensor("perm", [128, 128], bf, kind="ExternalInput")
    out_d = nc.dram_tensor("out", [NMCH, 128, RPB], bf,
                           kind="ExternalOutput")

    with tile.TileContext(nc) as tc:
        with (
            tc.tile_pool(name="consts", bufs=1) as consts,
            tc.tile_pool(name="wstream", bufs=3) as wstream,
            tc.tile_pool(name="qsb", bufs=3) as qsbp,
            tc.tile_pool(name="sq", bufs=2) as sqp,
            tc.tile_pool(name="t1", bufs=2) as t1p,
            tc.tile_pool(name="t2", bufs=2) as t2p,
            tc.tile_pool(name="outs", bufs=3) as outsp,
            tc.tile_pool(name="outq", bufs=6) as outqp,
            tc.tile_pool(name="ps", bufs=3, space="PSUM") as psp,
            tc.tile_pool(name="sw", bufs=2, space="PSUM") as swp,
            tc.tile_pool(name="rms", bufs=1, space="PSUM") as rmsp,
        ):
            xt = consts.tile([128, 12, RPB], bf)
            for kc3 in range(6):
                eng = (nc.sync, nc.scalar, nc.gpsimd)[kc3 % 3]
                eng.dma_start(xt[:, 2 * kc3:2 * kc3 + 2, :],
                              xt_d.ap()[:, 2 * kc3:2 * kc3 + 2, :])
            ct = consts.tile([128, RPB], bf)
            nc.scalar.dma_start(ct[:], ct_d.ap())
            st = consts.tile([128, RPB], bf)
            nc.gpsimd.dma_start(st[:], st_d.ap())
            perm = consts.tile([128, 128], bf)
            nc.sync.dma_start(perm[:], perm_d.ap())

            ones = consts.tile([128, 1], f32, name="ones")
            nc.vector.memset(ones[:], 1.0)
            ones1f = consts.tile([1, 128], f32, name="ones1f")
            nc.vector.memset(ones1f[:], 1.0)
            epsb = consts.tile([1, 1], f32, name="epsb")
            nc.vector.memset(epsb[:], EPS)

            # PE warmup (clock ramp) while the first DMAs land
            wsrc = consts.tile([128, RPB], bf, name="wsrc")
            nc.vector.memset(wsrc[:], 0.0)
            for wu in range(20):
                wp = psp.tile([128, RPB], f32, tag="ps", name="psw")
                nc.tensor.matmul(wp[:], wsrc[:, :128], wsrc[:],
                                 start=True, stop=True)

            # roped (unnormalized) T chunks, kept until the rms scale lands
            ropes = consts.tile([128, 12, RPB], bf, name="ropes")
            # pairwise sum-of-squares tree tiles (fp32)
            stree = {}
            for lv, n_ in (("a", 6), ("b", 3), ("c", 1)):
                for i_ in range(n_):
                    stree[(lv, i_)] = consts.tile([128, RPB], f32,
                                                  name=f"st{lv}{i_}")
            ssq = consts.tile([128, RPB], f32, name="ssq")
            rb = consts.tile([128, RPB], bf, name="rb")

            TT = nc.vector.tensor_tensor
            sq_tiles = {}
            pend_rope = []   # lag-1: perm matmul of chunk m after proj m+1

            def rope_chunk(m, qsb):
                sw = swp.tile([128, RPB], f32, tag="sw", name="sw")
                nc.tensor.matmul(sw[:], perm[:], qsb[:],
                                 start=True, stop=True)
                t1 = t1p.tile([128, RPB], bf, tag="t1", name="t1")
                TT(t1[:], qsb[:], ct[:], AluOpType.mult)
                t2 = t2p.tile([128, RPB], bf, tag="t2", name="t2")
                TT(t2[:], sw[:], st[:], AluOpType.mult)
                TT(ropes[:, m, :], t1[:], t2[:], AluOpType.add)

            for m in range(NMCH):
                wt = wstream.tile([128, 12, 128], bf, tag="w", name="wt")
                if m == 0:
                    # sliced so the first contraction chunks start early
                    for s6 in range(6):
                        eng = (nc.sync, nc.scalar, nc.gpsimd)[s6 % 3]
                        eng.dma_start(wt[:, 2 * s6:2 * s6 + 2, :],
                                      wt_d.ap()[0][:, 2 * s6:2 * s6 + 2, :])
                else:
                    eng = (nc.sync, nc.scalar)[m % 2]
                    eng.dma_start(wt[:], wt_d.ap()[m])
                ps = psp.tile([128, RPB], f32, tag="ps", name="ps")
                for kc in range(12):
                    nc.tensor.matmul(ps[:], wt[:, kc, :], xt[:, kc, :],
                                     start=(kc == 0), stop=(kc == 11))
                if m < 12:
                    qsb = qsbp.tile([128, RPB], bf, tag="q", name="qsb")
                    nc.scalar.activation(out=qsb[:], in_=ps[:], func=AF.Copy)
                    # fp32 squares + pairwise tree (VectorE)
                    sq = sqp.tile([128, RPB], f32, tag="s", name="sq")
                    TT(sq[:], qsb[:], qsb[:], AluOpType.mult)
                    sq_tiles[m] = sq
                    if m % 2 == 1:
                        TT(stree[("a", m // 2)][:], sq_tiles[m - 1][:],
                           sq[:], AluOpType.add)
                        del sq_tiles[m - 1], sq_tiles[m]
                    if m % 4 == 3:
                        TT(stree[("b", m // 4)][:],
                           stree[("a", m // 2 - 1)][:],
                           stree[("a", m // 2)][:], AluOpType.add)
                    if m == 7:
                        TT(stree[("c", 0)][:], stree[("b", 0)][:],
                           stree[("b", 1)][:], AluOpType.add)
                    if m == 11:
                        TT(ssq[:], stree[("c", 0)][:], stree[("b", 2)][:],
                           AluOpType.add)
                    pend_rope.append((m, qsb))
                    if len(pend_rope) > 1:
                        rope_chunk(*pend_rope.pop(0))
                else:
                    outv = outsp.tile([128, RPB], bf, tag="o", name="outv")
                    nc.vector.tensor_copy(outv[:], ps[:])
                    nc.gpsimd.dma_start(out_d.ap()[m], outv[:])

                if m == 12:
                    while pend_rope:
                        rope_chunk(*pend_rope.pop(0))
                    # rms finish: partition-reduce ssq via ones-matmul,
                    # sqrt(ms+eps), fast reciprocal, broadcast back; high
                    # priority so the scheduler doesn't defer it to the tail
                    with tc.high_priority():
                        sps = rmsp.tile([1, RPB], f32, tag="r", name="sps")
                        nc.tensor.matmul(sps[:], ones[:], ssq[:],
                                         start=True, stop=True)
                        rrow = consts.tile([1, RPB], f32, name="rrow")
                        nc.scalar.activation(out=rrow[:], in_=sps[:],
                                             func=AF.Sqrt, bias=epsb[:],
                                             scale=1.0 / DIM)
                        rinv = consts.tile([1, RPB], f32, name="rinv")
                        nc.vector.reciprocal_approx_fast(out=rinv[:],
                                                         in_=rrow[:])
                        rbp = rmsp.tile([128, RPB], f32, tag="rb",
                                        name="rbp")
                        nc.tensor.matmul(rbp[:], ones1f[:], rinv[:],
                                         start=True, stop=True)
                        nc.vector.tensor_copy(rb[:], rbp[:])
                if m == 13:
                    # all normalized T-chunk stores; high priority so the
                    # scheduler hides them behind the V chunks' PE stream
                    with tc.high_priority():
                        for m2 in range(12):
                            outq = outqp.tile([128, RPB], bf, tag="oq",
                                              name="outq")
                            TT(outq[:], ropes[:, m2, :], rb[:],
                               AluOpType.mult)
                            nc.gpsimd.dma_start(out_d.ap()[m2], outq[:])

    nc.finalize()
    return nc


# ---------------------------------------------------------------------------
# Launch 2 (2D-sharded): core (hg, qh) handles 3 heads x 780 queries x all
# 4680 keys.  Per-core KV DMA drops 28.7MB -> 7.2MB, QK/PV moving width
# rises to 512/268, denominators via a VectorE tree-reduce over the
# resident P tile + gpsimd partition_all_reduce, o3 normalized straight
# out of the PV PSUM bank, and the per-head-group partial out-projection
# (summed on the host) is interleaved into the second query block's
# QK/PV stream so it costs no wall time; only the last query block's
# out-proj rounds remain as tail.
HPC = 3            # heads per core
QPC = 780          # queries per core
QB = (512, 268)    # query blocks (one PSUM bank each)
NGR = 13           # 13 exp groups of 3 key chunks (last group = 1 chunk)


def _build_launch2():
    nc = bacc.Bacc("TRN2", target_bir_lowering=False, debug=False,
                   num_devices=NCORES, num_swdge_queues=4)
    f32, bf = mybir.dt.float32, mybir.dt.bfloat16

    qt_d = nc.dram_tensor("qt", [128, HPC, 784], bf, kind="ExternalInput")
    kt_d = nc.dram_tensor("kt", [HPC, 128, CACHE], bf, kind="ExternalInput")
    vt_d = nc.dram_tensor("vt", [HPC, 128, NKC, 128], bf, kind="ExternalInput")
    w2_d = nc.dram_tensor("w2", [128, HPC, 3, 512], bf, kind="ExternalInput")
    out_d = nc.dram_tensor("outp", [QPC, DIM], bf, kind="ExternalOutput")

    with tile.TileContext(nc) as tc:
        with (
            tc.tile_pool(name="consts", bufs=1) as consts,
            tc.tile_pool(name="kv", bufs=3) as kvp,
            tc.tile_pool(name="acc", bufs=1) as accp,
            tc.tile_pool(name="lp", bufs=2, space="PSUM") as lpp,
            tc.tile_pool(name="ops", bufs=1, space="PSUM") as opsp,
            tc.tile_pool(name="pop", bufs=1, space="PSUM") as popp,
            tc.tile_pool(name="outs", bufs=3) as outsp,
            tc.tile_pool(name="outq", bufs=6) as outqp,
        ):
            qt = consts.tile([128, HPC, 784], bf)
            # head-0 queries first (the very first QK gates on them)
            nc.sync.dma_start(qt[:, 0, :512], qt_d.ap()[:, 0, :512])
            nc.scalar.dma_start(qt[:, 0, 512:], qt_d.ap()[:, 0, 512:])
            for qh_ in range(1, HPC):
                nc.scalar.dma_start(qt[:, qh_, :], qt_d.ap()[:, qh_, :])
            w2 = consts.tile([128, HPC, 3, 512], bf)
            nc.scalar.dma_start(w2[:], w2_d.ap())
            o3 = consts.tile([128, HPC, 784], bf)  # normalized o^T per head
            # PE warmup: ~3us of matmul trips the clock gate to full rate
            # while the first K/Q DMAs land
            wsrc = consts.tile([128, 512], bf, name="wsrc")
            nc.vector.memset(wsrc[:], 0.0)
            for wu in range(14):
                wp = lpp.tile([128, 3, 512], f32, tag="lp", name="lpw")
                nc.tensor.matmul(wp[:, 0, :], wsrc[:, :128], wsrc[:],
                                 start=True, stop=True)

            # den tree scratch (bf16 pairwise tree over the resident P tile)
            acc1 = accp.tile([128, 18, 512], bf, name="acc1")
            acc2 = accp.tile([128, 9, 512], bf, name="acc2")
            acc3 = accp.tile([128, 4, 512], bf, name="acc3")
            acc4 = accp.tile([128, 2, 512], bf, name="acc4")
            acc5 = accp.tile([128, 512], bf, name="acc5")
            acc6 = accp.tile([128, 512], bf, name="acc6")
            den = accp.tile([128, 512], bf, name="den")
            sden = accp.tile([128, 512], f32, name="sden")
            sinv = accp.tile([128, 512], f32, name="sinv")
            # staging for the PV accumulator so its PSUM bank frees long
            # before the den chain finishes (double-buffered)
            o3us = [accp.tile([128, 512], f32, name=f"o3u{i}")
                    for i in range(2)]

            # double-buffered resident P tiles; the tail chunk's garbage
            # rows [72:128] are zeroed once so the den tree reads zeros
            # (the PV matmul is safe regardless: V rows 72: are zero)
            pts = [consts.tile([128, NKC, 512], bf, name=f"pt{i}")
                   for i in range(2)]
            # (partition base must be 32-aligned: clear the whole slot;
            # exp overwrites rows [:72] every iteration)
            nc.vector.memset(pts[0][:, NKC - 1, :], 0.0)
            nc.vector.memset(pts[1][:, NKC - 1, :], 0.0)

            # software pipeline: each entry is (pv_closure, finish_closure);
            # the last two PV groups of every head are deliberately held
            # back and drained at the START of the next head, filling the
            # pipeline-fill bubble while that head's first exp is in flight
            pending_pv = []

            def pop_pv():
                pv, fin = pending_pv.pop(0)
                pv()
                if fin is not None:
                    fin()

            # all K/V loads upfront (resident for both query blocks); the
            # triggers sit ahead of every partition_all_reduce in the
            # gpsimd FIFO so DMAs never wait on compute
            kts, vts = {}, {}
            for h in range(HPC):
                kts[h] = kvp.tile([128, CACHE], bf, tag="kt",
                                  name=f"ktile{h}")
                vts[h] = kvp.tile([128, NKC, 128], bf, tag="vt",
                                  name=f"vtile{h}")
            nc.sync.dma_start(kts[0][:, :128], kt_d.ap()[0][:, :128])
            nc.sync.dma_start(kts[0][:, 128:256], kt_d.ap()[0][:, 128:256])
            nc.gpsimd.dma_start(vts[0][:, :6, :], vt_d.ap()[0][:, :6, :])
            nc.sync.dma_start(kts[0][:, 256:1536], kt_d.ap()[0][:, 256:1536])
            nc.sync.dma_start(kts[0][:, 1536:], kt_d.ap()[0][:, 1536:])
            nc.gpsimd.dma_start(vts[0][:, 6:, :], vt_d.ap()[0][:, 6:, :])
            for h in range(1, HPC):
                nc.sync.dma_start(kts[h][:], kt_d.ap()[h])
                nc.gpsimd.dma_start(vts[h][:], vt_d.ap()[h])

            def out_round(r0, r1, nf):
                rs = r1 - r0
                po = popp.tile([128, 512], f32, tag="po", name="po")
                for h in range(HPC):
                    nc.tensor.matmul(
                        po[:rs, :],
                        o3[:, h, r0:r1],
                        w2[:, h, nf, :],
                        start=(h == 0), stop=(h == HPC - 1))
                outf = outsp.tile([128, 512], bf, tag="of", name="of")
                nc.vector.tensor_copy(outf[:rs, :], po[:rs, :])
                deng = (nc.sync, nc.scalar, nc.gpsimd)[nf]
                deng.dma_start(
                    out_d.ap()[r0:r1, nf * 512:(nf + 1) * 512],
                    outf[:rs, :])

            # qb0's out-proj rounds, interleaved into qb1's stream
            deferred = [(r0, r1, nf)
                        for (r0, r1) in ((0, 128), (128, 256),
                                         (256, 384), (384, 512))
                        for nf in range(3)]

            TT = nc.vector.tensor_tensor

            def tree_levels(qw):
                TT(acc2[:, :, :qw], acc1[:, 0:9, :qw],
                   acc1[:, 9:18, :qw], AluOpType.add)
                TT(acc3[:, :, :qw], acc2[:, 0:4, :qw],
                   acc2[:, 4:8, :qw], AluOpType.add)
                TT(acc4[:, :, :qw], acc3[:, 0:2, :qw],
                   acc3[:, 2:4, :qw], AluOpType.add)
                TT(acc5[:, :qw], acc4[:, 0, :qw], acc4[:, 1, :qw],
                   AluOpType.add)
                TT(acc6[:, :qw], acc5[:, :qw], acc2[:, 8, :qw],
                   AluOpType.add)

            for qb in range(2):
                qw = QB[qb]
                q0 = 512 * qb
                for h in range(HPC):
                    kt, vt = kts[h], vts[h]
                    last = (qb == 1 and h == HPC - 1)
                    o3u = o3us[(qb * HPC + h) % 2]
                    opsum = opsp.tile([128, 512], f32, tag="opsum",
                                      name="opsum")
                    pt = pts[(qb * HPC + h) % 2]
                    for g in range(NGR):
                        jj = 3 * g
                        nch = min(3, NKC - jj)
                        pw = 128 if nch == 3 else TAIL  # exp partition rows
                        lp = lpp.tile([128, 3, 512], f32, tag="lp", name="lp")
                        for u in range(nch):
                            j = jj + u
                            kw = TAIL if j == NKC - 1 else 128
                            nc.tensor.matmul(
                                lp[:kw, u, :qw],
                                kt[:, j * 128:j * 128 + kw],
                                qt[:, h, q0:q0 + qw],
                                start=True, stop=True)
                        nc.scalar.activation(
                            out=pt[:pw, jj:jj + nch, :qw],
                            in_=lp[:pw, :nch, :qw], func=AF.Exp)

                        def pv_group(jj=jj, nch=nch, pt=pt, vt=vt,
                                     opsum=opsum, qw=qw):
                            for u in range(nch):
                                j = jj + u
                                nc.tensor.matmul(
                                    opsum[:, :qw],
                                    vt[:, j, :],
                                    pt[:, j, :qw],
                                    start=(j == 0), stop=(j == NKC - 1))

                        if g < NGR - 1:
                            fin = None
                        else:
                            def fin(h=h, q0=q0, qw=qw, o3u=o3u, opsum=opsum):
                                # free the PV PSUM bank right away, then
                                # normalize; the reduce ran during g12
                                nc.vector.tensor_copy(o3u[:, :qw],
                                                      opsum[:, :qw])
                                nc.vector.reciprocal_approx_fast(
                                    out=sinv[:, :qw], in_=sden[:, :qw])
                                nc.vector.tensor_tensor(
                                    o3[:, h, q0:q0 + qw], o3u[:, :qw],
                                    sinv[:, :qw], AluOpType.mult)
                        pending_pv.append((pv_group, fin))
                        if g < NGR - 2:
                            while len(pending_pv) > 1:
                                pop_pv()

                        # den tree emitted piecewise so only the tiny final
                        # add + partition-reduce trail the last exp
                        if 6 <= g <= 11:
                            u0 = 3 * (g - 6)
                            TT(acc1[:, u0:u0 + 3, :qw],
                               pt[:, u0:u0 + 3, :qw],
                               pt[:, u0 + 18:u0 + 21, :qw],
                               AluOpType.add)
                            if g == 11:
                                tree_levels(qw)
                        elif g == NGR - 1:
                            TT(den[:, :qw], acc6[:, :qw],
                               pt[:, 36, :qw], AluOpType.add)
                            nc.gpsimd.partition_all_reduce(
                                sden[:, :qw], den[:, :qw], 128,
                                bass_isa.ReduceOp.add)
                        if qb == 1 and deferred and (
                                (h == 1 and g >= 6) or (h == 2 and g < 6)):
                            out_round(*deferred.pop(0))

            while pending_pv:
                pop_pv()
            # drain any undeferred qb0 rounds, then qb1's tail rounds
            # (double-buffered on the now-idle QK PSUM banks)
            tail_rounds = list(deferred)
            deferred.clear()
            tail_rounds += [(r0, r1, nf)
                            for (r0, r1) in ((512, 640), (640, 768),
                                             (768, 780))
                            for nf in range(3)]
            for (r0, r1, nf) in tail_rounds:
                rs = r1 - r0
                po = lpp.tile([128, 3, 512], f32, tag="lp", name="lp")
                for h in range(HPC):
                    nc.tensor.matmul(
                        po[:rs, 0, :],
                        o3[:, h, r0:r1],
                        w2[:, h, nf, :],
                        start=(h == 0), stop=(h == HPC - 1))
                outf = outsp.tile([128, 512], bf, tag="of", name="of")
                nc.vector.tensor_copy(outf[:rs, :], po[:rs, 0, :])
                deng = (nc.sync, nc.scalar, nc.gpsimd)[nf]
                deng.dma_start(
                    out_d.ap()[r0:r1, nf * 512:(nf + 1) * 512],
                    outf[:rs, :])

    nc.finalize()
    return nc


# ---------------------------------------------------------------------------
def _cache_plan(current_start, global_end_index, local_end_index, s, kv_size,
                frame_seqlen):
    """Numpy re-implementation of the reference's cache roll/update/window
    logic, tracking only *indices*: returns (old_cache_rows, new_rows) such
    that the attended key set == cache[old_cache_rows] ++ new[new_rows]."""
    current_end = current_start + s
    sink_tokens = SINK_SIZE * frame_seqlen

    # each cache slot: kind 0 -> original cache row idx, kind 1 -> new row idx
    kind = np.zeros(kv_size, dtype=np.int64)
    idx = np.arange(kv_size, dtype=np.int64)

    if (LOCAL_ATTN_SIZE != -1 and current_end > global_end_index
            and s + local_end_index > kv_size):
        num_evicted = s + local_end_index - kv_size
        num_rolled = local_end_index - num_evicted - sink_tokens
        src0 = sink_tokens + num_evicted
        kind[sink_tokens:sink_tokens + num_rolled] = \
            kind[src0:src0 + num_rolled]
        idx[sink_tokens:sink_tokens + num_rolled] = \
            idx[src0:src0 + num_rolled]
        new_local_end = (local_end_index + current_end - global_end_index
                         - num_evicted)
    else:
        new_local_end = local_end_index + current_end - global_end_index
    local_start = new_local_end - s
    is_recompute = (current_end <= global_end_index) and (current_start > 0)
    write_start = max(local_start, sink_tokens) if is_recompute \
        else local_start
    off = max(0, write_start - local_start)
    wl = max(0, new_local_end - write_start)
    if wl > 0:
        kind[write_start:new_local_end] = 1
        idx[write_start:new_local_end] = off + np.arange(wl)

    if sink_tokens > 0:
        budget = MAX_ATTN - sink_tokens
        if budget > 0:
            lo = max(sink_tokens, new_local_end - budget)
            sel = np.concatenate([np.arange(sink_tokens),
                                  np.arange(lo, new_local_end)])
        else:
            sel = np.arange(sink_tokens)
    else:
        ws = max(0, new_local_end - MAX_ATTN)
        sel = np.arange(ws, new_local_end)

    k_kind, k_idx = kind[sel], idx[sel]
    old_rows = k_idx[k_kind == 0]
    new_rows = k_idx[k_kind == 1]
    return old_rows, new_rows


def _rope_tables_head(freqs_real, freqs_imag, f, h, w, start_frame,
                      gq, gk):
    """Per-head (S,128) cos table and sign-folded sin table, with the
    per-head g block folded in (g must be identical across heads)."""
    c = HD // 2  # 64
    c0 = c - 2 * (c // 3)
    c1 = c // 3
    fr = np.asarray(freqs_real, np.float32)
    fi = np.asarray(freqs_imag, np.float32)
    s = f * h * w
    assert s == S
    fidx = np.arange(s) // (h * w)
    hidx = (np.arange(s) // w) % h
    widx = np.arange(s) % w
    fr_pos = np.concatenate([
        fr[start_frame + fidx][:, :c0],
        fr[hidx][:, c0:c0 + c1],
        fr[widx][:, c0 + c1:c0 + 2 * c1],
    ], axis=1)  # (S, 64)
    fi_pos = np.concatenate([
        fi[start_frame + fidx][:, :c0],
        fi[hidx][:, c0:c0 + c1],
        fi[widx][:, c0 + c1:c0 + 2 * c1],
    ], axis=1)
    C1 = np.repeat(fr_pos, 2, axis=1)              # (S, 128)
    Sg = np.empty((s, HD), np.float32)
    Sg[:, 0::2] = -fi_pos                          # y_even = xe*c - xo*si
    Sg[:, 1::2] = fi_pos                           # y_odd  = xo*c + xe*si
    gq = np.asarray(gq, np.float32).reshape(NH, HD)
    gk = np.asarray(gk, np.float32).reshape(NH, HD)
    assert np.array_equal(gq, np.broadcast_to(gq[0], gq.shape)) and \
        np.array_equal(gk, np.broadcast_to(gk[0], gk.shape)), \
        "g must be identical across heads for shared rope tables"
    gqb, gkb = gq[0], gk[0]
    gqs = gqb.reshape(-1, 2)[:, ::-1].reshape(-1)
    gks = gkb.reshape(-1, 2)[:, ::-1].reshape(-1)
    return (C1 * gqb[None, :], Sg * gqs[None, :],
            C1 * gkb[None, :], Sg * gks[None, :])


# ---------------------------------------------------------------------------
def kernel(x, cache_k, cache_v, freqs_real, freqs_imag,
           wq, bq, wk, bk, wv, bv, wo, bo, gq, gk,
           f_frames, height, width, current_start, global_end_index,
           local_end_index):
    global LAST_RUNS
    LAST_RUNS = []

    x = np.asarray(x, np.float32)
    cache_k = np.asarray(cache_k, np.float32)
    cache_v = np.asarray(cache_v, np.float32)
    wq = np.asarray(wq, np.float32)
    wk = np.asarray(wk, np.float32)
    wv = np.asarray(wv, np.float32)
    wo = np.asarray(wo, np.float32)
    bo = np.asarray(bo, np.float32)
    f = int(f_frames)
    h = int(height)
    w = int(width)
    current_start = int(current_start)
    global_end_index = int(global_end_index)
    local_end_index = int(local_end_index)

    assert x.shape == (1, S, DIM)
    for b in (bq, bk, bv):
        assert not np.any(np.asarray(b)), "nonzero qkv bias unsupported"

    frame_seqlen = h * w
    start_frame = current_start // frame_seqlen

    # ---- launch 1: projections + RMS + RoPE (pair-split, transposed) ----
    Cq, Sq, Ck, Sk = _rope_tables_head(freqs_real, freqs_imag, f, h, w,
                                       start_frame, gq, gk)
    att_sc = 1.0 / float(np.sqrt(HD))   # q-only scale, folded into tables

    def _wtiles(W):
        return np.ascontiguousarray(
            W.reshape(12, 128, NMCH, 128).transpose(2, 1, 0, 3)).astype(BF16)

    wtA = _wtiles(np.concatenate([wq, wv[:, :768]], axis=1))
    wtB = _wtiles(np.concatenate([wk, wv[:, 768:]], axis=1))
    xT = x[0].T.astype(BF16)                                # (1536, 1560)
    permM = np.zeros((128, 128), BF16)
    idx128 = np.arange(128)
    permM[idx128 ^ 1, idx128] = 1.0

    ctA = np.ascontiguousarray((Cq * att_sc).T).astype(BF16)  # (128, S)
    stA = np.ascontiguousarray((Sq * att_sc).T).astype(BF16)
    ctB = np.ascontiguousarray(Ck.T).astype(BF16)
    stB = np.ascontiguousarray(Sk.T).astype(BF16)

    nc1 = _CACHED.get("l1")
    if nc1 is None:
        nc1 = _CACHED["l1"] = _build_launch1()

    in_maps1 = []
    for c in range(NCORES):
        p = c // 2
        r0, r1 = p * RPB, (p + 1) * RPB
        xt_c = np.ascontiguousarray(
            xT[:, r0:r1].reshape(12, 128, RPB).transpose(1, 0, 2))
        if c % 2 == 0:
            in_maps1.append({"xt": xt_c, "wt": wtA, "perm": permM,
                             "ct": ctA[:, r0:r1].copy(),
                             "st": stA[:, r0:r1].copy()})
        else:
            in_maps1.append({"xt": xt_c, "wt": wtB, "perm": permM,
                             "ct": ctB[:, r0:r1].copy(),
                             "st": stB[:, r0:r1].copy()})
    res1 = bass_utils.run_bass_kernel_spmd(nc1, in_maps1,
                                           core_ids=list(range(NCORES)))
    LAST_RUNS.append(res1)
    # transposed outputs: [18, 128, 390] per core -> head-major T tensors
    QT = np.concatenate(
        [res1.results[2 * p]["out"][:12] for p in range(4)], axis=2)
    KnewT = np.concatenate(
        [res1.results[2 * p + 1]["out"][:12] for p in range(4)], axis=2)
    VnewT = np.concatenate(
        [np.concatenate([res1.results[2 * p]["out"][12:],
                         res1.results[2 * p + 1]["out"][12:]], axis=0)
         for p in range(4)], axis=2)          # (12, 128, 1560)

    # ---- host glue: effective K/V assembly ----
    old_rows, new_rows = _cache_plan(current_start, global_end_index,
                                     local_end_index, S, cache_k.shape[1],
                                     frame_seqlen)
    n_keys = len(old_rows) + len(new_rows)
    assert n_keys == CACHE, f"unexpected key count {n_keys}"

    n_old = len(old_rows)
    ktT = np.empty((NH, HD, CACHE), BF16)
    ktT[:, :, :n_old] = cache_k[0, old_rows].astype(BF16).transpose(1, 2, 0)
    ktT[:, :, n_old:] = KnewT[:, :, new_rows]
    kt = ktT
    V_pad = np.zeros((NKC * 128, NH, HD), BF16)
    V_pad[:n_old] = cache_v[0, old_rows].astype(BF16)
    V_pad[n_old:CACHE] = VnewT[:, :, new_rows].transpose(2, 0, 1)
    vt = np.ascontiguousarray(
        V_pad.reshape(NKC, 128, NH, HD).transpose(2, 1, 0, 3))
    w2 = np.ascontiguousarray(
        wo.reshape(12, 128, 3, 512).transpose(1, 0, 2, 3)).astype(BF16)

    nc2 = _CACHED.get("l2")
    if nc2 is None:
        nc2 = _CACHED["l2"] = _build_launch2()

    in_maps2 = []
    for c in range(NCORES):
        hg, qh = c // 2, c % 2
        h0 = hg * HPC
        r0, r1 = qh * QPC, (qh + 1) * QPC
        qt_c = np.zeros((128, HPC, 784), BF16)
        qt_c[:, :, :QPC] = QT[h0:h0 + HPC, :, r0:r1].transpose(1, 0, 2)
        in_maps2.append({
            "qt": qt_c,
            "kt": np.ascontiguousarray(kt[h0:h0 + HPC]),
            "vt": np.ascontiguousarray(vt[h0:h0 + HPC]),
            "w2": np.ascontiguousarray(w2[:, h0:h0 + HPC]),
        })
    res2 = bass_utils.run_bass_kernel_spmd(nc2, in_maps2,
                                           core_ids=list(range(NCORES)))
    LAST_RUNS.append(res2)

    out = np.zeros((S, DIM), np.float32)
    for c in range(NCORES):
        hg, qh = c // 2, c % 2
        out[qh * QPC:(qh + 1) * QPC] += \
            res2.results[c]["outp"].astype(np.float32)
    out += bo.reshape(1, DIM)
    return out.reshape(1, S, DIM)



# revision 36
# speedup vs baseline: 1.1954x; 1.1954x over previous
"""Trainium2 Bass kernel for nn_CausalWanSelfAttention_45904610460041.

Strategy (8 NeuronCores, full I/O):
  Launch 1 (pair-split, transposed projections): cores 2p/2p+1 share 390
    rows; core 2p computes wq + wv[:, :768], core 2p+1 wk + wv[:, 768:]
    (column-complete, so RMS stays core-local; per-core weight DMA
    ~7.1MB).  The GEMM runs transposed (out^T = W^T x^T): M = 18 exact
    128-col chunks (zero M-padding), N = 390 rows in a single PSUM bank.
    RoPE's pair-swap is a PE permutation matmul; RMS sum-of-squares is a
    bf16 square + fp32 pairwise tree + ones-matmul partition reduce +
    Sqrt + fast reciprocal + ones-broadcast matmul, applied fused with
    the output cast.  T chunks run first so the normalize tail hides
    under the V chunks' PE stream.
  Host glue: applies the (numpy, index-only) KV-cache roll/update/window
    logic and assembles the effective 4680-key K^T / V tensors per head
    (cache rows pass through; softmax is permutation-invariant so key
    order is free).  Partial out-projection sums + bias also on host.
  Launch 2 (2D head x query sharded): core (hg, qh) takes 3 heads x 780
    queries x all 4680 keys (per-core KV DMA 7.2MB).  Query blocks of
    512/268, logits^T = K^T-chunk . Q (keys on partitions), one exp per
    3-chunk PSUM group on ScalarE (the hard floor: ~86k lane-elems),
    P.V accumulated per block.  The last two PV groups of each head are
    held back and drained at the next head's start, filling the
    exp-latency pipeline bubble.  Softmax denominators via a bf16
    pairwise tree emitted PIECEWISE while groups complete (acc1 during
    g6-g11, levels at g11, final add + gpsimd partition_all_reduce at
    g12), so each head's chain is short: PSUM-stage copy + fast
    reciprocal + multiply.  The per-head-group partial out-projection is
    interleaved into the second query block's stream (the scheduler
    hoists rounds right behind the o3 normalizes); only the last block's
    9 rounds trail, double-buffered on the then-idle QK PSUM banks.

  Measured on trn2 (8 cores, fast clock state): launch1 ~70us + launch2
  ~151us = ~224us total HW exec (run-to-run chip clock variance ~15%);
  relative error vs the fp32 reference ~7.1e-3 (matmul operands bf16,
  fp32 accumulation).
"""

import os
import sys

for _p in ("/opt/trn_rl_repo",):
    if os.path.isdir(_p) and _p not in sys.path:
        sys.path.insert(0, _p)

import numpy as np
import ml_dtypes

import concourse.bass as bass
import concourse.tile as tile
from concourse import bacc
from concourse import mybir
from concourse import bass_utils
from concourse import bass_isa
from concourse.alu_op_type import AluOpType

BF16 = ml_dtypes.bfloat16
AF = mybir.ActivationFunctionType

# ---------------------------------------------------------------------------
# Problem constants (fixed by the input specs).
S = 1560          # query/new-token sequence length
DIM = 1536
NH = 12
HD = 128
CACHE = 4680      # kv cache length == effective attention keys here
NCORES = 8
RPC = S // NCORES  # 195 rows (queries) per core
EPS = 1e-6
LOCAL_ATTN_SIZE = 3
SINK_SIZE = 1
MAX_ATTN = 32760 if LOCAL_ATTN_SIZE == -1 else LOCAL_ATTN_SIZE * S

NKC = (CACHE + 127) // 128      # 37 key chunks
TAIL = CACHE - (NKC - 1) * 128  # 72 keys in the tail chunk

# Row-chunk split of the 195 per-core rows into <=128-partition chunks.
RCHUNKS = [(0, 128), (128, 195)]

_CACHED = {}
LAST_RUNS = []  # BassKernelResults of the most recent kernel() call


# ---------------------------------------------------------------------------
# Launch 1 (pair-split, transposed): cores 2p/2p+1 share rows
# [390p, 390p+390).  Core 2p computes all of wq plus wv[:, :768]; core
# 2p+1 all of wk plus wv[:, 768:] (column-complete -> RMS stays local).
# The matmul is TRANSPOSED (out^T = W^T x^T): M = output columns (18
# exact 128-chunks, zero M-padding), N = 390 rows (single PSUM bank),
# so PE streams 18*12*390 columns instead of the row-major 4*12*2304.
# RoPE's pair-swap is a PE permutation matmul; the RMS sum-of-squares
# partition reduction is a ones-vector matmul; rsqrt is Sqrt+fast-recip;
# the per-row 1/rms scale is broadcast back with a second tiny matmul
# and applied fused with the output cast.  T (q/k) chunks run first so
# the normalize tail overlaps the V chunks' PE stream.  The attention
# scale and g vectors are folded into the host-built tables (g must be
# identical across heads -- asserted; it is all-ones here).
RPB = 390                 # rows per core (pair rows)
NMCH = 18                 # output column chunks (12 T + 6 V)


def _build_launch1():
    nc = bacc.Bacc("TRN2", target_bir_lowering=False, debug=False,
                   num_devices=NCORES, num_swdge_queues=4)
    f32, bf = mybir.dt.float32, mybir.dt.bfloat16

    xt_d = nc.dram_tensor("xt", [128, 12, RPB], bf, kind="ExternalInput")
    wt_d = nc.dram_tensor("wt", [NMCH, 128, 12, 128], bf,
                          kind="ExternalInput")
    ct_d = nc.dram_tensor("ct", [128, RPB], bf, kind="ExternalInput")
    st_d = nc.dram_tensor("st", [128, RPB], bf, kind="ExternalInput")
    perm_d = nc.dram_tensor("perm", [128, 128], bf, kind="ExternalInput")
    out_d = nc.dram_tensor("out", [NMCH, 128, RPB], bf,
                           kind="ExternalOutput")

    with tile.TileContext(nc) as tc:
        with (
            tc.tile_pool(name="consts", bufs=1) as consts,
            tc.tile_pool(name="wstream", bufs=3) as wstream,
            tc.tile_pool(name="qsb", bufs=3) as qsbp,
            tc.tile_pool(name="sq", bufs=2) as sqp,
            tc.tile_pool(name="t1", bufs=2) as t1p,
            tc.tile_pool(name="t2", bufs=2) as t2p,
            tc.tile_pool(name="outs", bufs=3) as outsp,
            tc.tile_pool(name="outq", bufs=6) as outqp,
            tc.tile_pool(name="ps", bufs=3, space="PSUM") as psp,
            tc.tile_pool(name="sw", bufs=2, space="PSUM") as swp,
            tc.tile_pool(name="rms", bufs=1, space="PSUM") as rmsp,
        ):
            xt = consts.tile([128, 12, RPB], bf)
            for kc3 in range(6):
                eng = (nc.sync, nc.scalar, nc.gpsimd)[kc3 % 3]
                eng.dma_start(xt[:, 2 * kc3:2 * kc3 + 2, :],
                              xt_d.ap()[:, 2 * kc3:2 * kc3 + 2, :])
            ct = consts.tile([128, RPB], bf)
            nc.scalar.dma_start(ct[:], ct_d.ap())
            st = consts.tile([128, RPB], bf)
            nc.gpsimd.dma_start(st[:], st_d.ap())
            perm = consts.tile([128, 128], bf)
            nc.sync.dma_start(perm[:], perm_d.ap())

            ones = consts.tile([128, 1], f32, name="ones")
            nc.vector.memset(ones[:], 1.0)
            ones1f = consts.tile([1, 128], f32, name="ones1f")
            nc.vector.memset(ones1f[:], 1.0)
            epsb = consts.tile([1, 1], f32, name="epsb")
            nc.vector.memset(epsb[:], EPS)

            # PE warmup (clock ramp) while the first DMAs land
            wsrc = consts.tile([128, RPB], bf, name="wsrc")
            nc.vector.memset(wsrc[:], 0.0)
            for wu in range(12):
                wp = psp.tile([128, RPB], f32, tag="ps", name="psw")
                nc.tensor.matmul(wp[:], wsrc[:, :128], wsrc[:],
                                 start=True, stop=True)

            # roped (unnormalized) T chunks, kept until the rms scale lands
            ropes = consts.tile([128, 12, RPB], bf, name="ropes")
            # pairwise sum-of-squares tree tiles (fp32)
            stree = {}
            for lv, n_ in (("a", 6), ("b", 3), ("c", 1)):
                for i_ in range(n_):
                    stree[(lv, i_)] = consts.tile([128, RPB], f32,
                                                  name=f"st{lv}{i_}")
            ssq = consts.tile([128, RPB], f32, name="ssq")
            rb = consts.tile([128, RPB], bf, name="rb")

            TT = nc.vector.tensor_tensor
            sq_tiles = {}
            pend_rope = []   # lag-1: perm matmul of chunk m after proj m+1

            def rope_chunk(m, qsb):
                sw = swp.tile([128, RPB], f32, tag="sw", name="sw")
                nc.tensor.matmul(sw[:], perm[:], qsb[:],
                                 start=True, stop=True)
                t1 = t1p.tile([128, RPB], bf, tag="t1", name="t1")
                TT(t1[:], qsb[:], ct[:], AluOpType.mult)
                t2 = t2p.tile([128, RPB], bf, tag="t2", name="t2")
                TT(t2[:], sw[:], st[:], AluOpType.mult)
                TT(ropes[:, m, :], t1[:], t2[:], AluOpType.add)

            for m in range(NMCH):
                wt = wstream.tile([128, 12, 128], bf, tag="w", name="wt")
                if m == 0:
                    # sliced so the first contraction chunks start early
                    for s6 in range(6):
                        eng = (nc.sync, nc.scalar, nc.gpsimd)[s6 % 3]
                        eng.dma_start(wt[:, 2 * s6:2 * s6 + 2, :],
                                      wt_d.ap()[0][:, 2 * s6:2 * s6 + 2, :])
                else:
                    eng = (nc.sync, nc.scalar, nc.gpsimd)[m % 3]
                    eng.dma_start(wt[:], wt_d.ap()[m])
                ps = psp.tile([128, RPB], f32, tag="ps", name="ps")
                for kc in range(12):
                    nc.tensor.matmul(ps[:], wt[:, kc, :], xt[:, kc, :],
                                     start=(kc == 0), stop=(kc == 11))
                if m < 12:
                    qsb = qsbp.tile([128, RPB], bf, tag="q", name="qsb")
                    nc.scalar.activation(out=qsb[:], in_=ps[:], func=AF.Copy)
                    # fp32 squares + pairwise tree (VectorE)
                    sq = sqp.tile([128, RPB], f32, tag="s", name="sq")
                    TT(sq[:], qsb[:], qsb[:], AluOpType.mult)
                    sq_tiles[m] = sq
                    if m % 2 == 1:
                        TT(stree[("a", m // 2)][:], sq_tiles[m - 1][:],
                           sq[:], AluOpType.add)
                        del sq_tiles[m - 1], sq_tiles[m]
                    if m % 4 == 3:
                        TT(stree[("b", m // 4)][:],
                           stree[("a", m // 2 - 1)][:],
                           stree[("a", m // 2)][:], AluOpType.add)
                    if m == 7:
                        TT(stree[("c", 0)][:], stree[("b", 0)][:],
                           stree[("b", 1)][:], AluOpType.add)
                    if m == 11:
                        TT(ssq[:], stree[("c", 0)][:], stree[("b", 2)][:],
                           AluOpType.add)
                    pend_rope.append((m, qsb))
                    if len(pend_rope) > 1:
                        rope_chunk(*pend_rope.pop(0))
                else:
                    outv = outsp.tile([128, RPB], bf, tag="o", name="outv")
                    nc.vector.tensor_copy(outv[:], ps[:])
                    eng2 = (nc.sync, nc.scalar, nc.gpsimd)[m % 3]
                    eng2.dma_start(out_d.ap()[m], outv[:])

                if m == 12:
                    while pend_rope:
                        rope_chunk(*pend_rope.pop(0))
                    # rms finish: partition-reduce ssq via ones-matmul,
                    # sqrt(ms+eps), fast reciprocal, broadcast back; high
                    # priority so the scheduler doesn't defer it to the tail
                    with tc.high_priority():
                        sps = rmsp.tile([1, RPB], f32, tag="r", name="sps")
                        nc.tensor.matmul(sps[:], ones[:], ssq[:],
                                         start=True, stop=True)
                        rrow = consts.tile([1, RPB], f32, name="rrow")
                        nc.scalar.activation(out=rrow[:], in_=sps[:],
                                             func=AF.Sqrt, bias=epsb[:],
                                             scale=1.0 / DIM)
                        rinv = consts.tile([1, RPB], f32, name="rinv")
                        nc.vector.reciprocal_approx_fast(out=rinv[:],
                                                         in_=rrow[:])
                        rbp = rmsp.tile([128, RPB], f32, tag="rb",
                                        name="rbp")
                        nc.tensor.matmul(rbp[:], ones1f[:], rinv[:],
                                         start=True, stop=True)
                        nc.vector.tensor_copy(rb[:], rbp[:])
                if m == 13:
                    # all normalized T-chunk stores; high priority so the
                    # scheduler hides them behind the V chunks' PE stream
                    with tc.high_priority():
                        for m2 in range(12):
                            outq = outqp.tile([128, RPB], bf, tag="oq",
                                              name="outq")
                            TT(outq[:], ropes[:, m2, :], rb[:],
                               AluOpType.mult)
                            eng3 = (nc.gpsimd, nc.sync, nc.scalar)[m2 % 3]
                            eng3.dma_start(out_d.ap()[m2], outq[:])

    nc.finalize()
    return nc


# ---------------------------------------------------------------------------
# Launch 2 (2D-sharded): core (hg, qh) handles 3 heads x 780 queries x all
# 4680 keys.  Per-core KV DMA drops 28.7MB -> 7.2MB, QK/PV moving width
# rises to 512/268, denominators via a VectorE tree-reduce over the
# resident P tile + gpsimd partition_all_reduce, o3 normalized straight
# out of the PV PSUM bank, and the per-head-group partial out-projection
# (summed on the host) is interleaved into the second query block's
# QK/PV stream so it costs no wall time; only the last query block's
# out-proj rounds remain as tail.
HPC = 3            # heads per core
QPC = 780          # queries per core
QB = (512, 268)    # query blocks (one PSUM bank each)
NGR = 13           # 13 exp groups of 3 key chunks (last group = 1 chunk)


def _build_launch2():
    nc = bacc.Bacc("TRN2", target_bir_lowering=False, debug=False,
                   num_devices=NCORES, num_swdge_queues=4)
    f32, bf = mybir.dt.float32, mybir.dt.bfloat16

    qt_d = nc.dram_tensor("qt", [128, HPC, 784], bf, kind="ExternalInput")
    kt_d = nc.dram_tensor("kt", [HPC, 128, CACHE], bf, kind="ExternalInput")
    vt_d = nc.dram_tensor("vt", [HPC, 128, NKC, 128], bf, kind="ExternalInput")
    w2_d = nc.dram_tensor("w2", [128, HPC, 3, 512], bf, kind="ExternalInput")
    out_d = nc.dram_tensor("outp", [QPC, DIM], bf, kind="ExternalOutput")

    with tile.TileContext(nc) as tc:
        with (
            tc.tile_pool(name="consts", bufs=1) as consts,
            tc.tile_pool(name="kv", bufs=3) as kvp,
            tc.tile_pool(name="acc", bufs=1) as accp,
            tc.tile_pool(name="lp", bufs=2, space="PSUM") as lpp,
            tc.tile_pool(name="ops", bufs=1, space="PSUM") as opsp,
            tc.tile_pool(name="pop", bufs=1, space="PSUM") as popp,
            tc.tile_pool(name="outs", bufs=3) as outsp,
            tc.tile_pool(name="outq", bufs=6) as outqp,
        ):
            qt = consts.tile([128, HPC, 784], bf)
            # head-0 queries first (the very first QK gates on them)
            nc.sync.dma_start(qt[:, 0, :512], qt_d.ap()[:, 0, :512])
            nc.scalar.dma_start(qt[:, 0, 512:], qt_d.ap()[:, 0, 512:])
            for qh_ in range(1, HPC):
                nc.scalar.dma_start(qt[:, qh_, :], qt_d.ap()[:, qh_, :])
            w2 = consts.tile([128, HPC, 3, 512], bf)
            nc.scalar.dma_start(w2[:], w2_d.ap())
            o3 = consts.tile([128, HPC, 784], bf)  # normalized o^T per head
            # PE warmup: ~3us of matmul trips the clock gate to full rate
            # while the first K/Q DMAs land
            wsrc = consts.tile([128, 512], bf, name="wsrc")
            nc.vector.memset(wsrc[:], 0.0)
            for wu in range(10):
                wp = lpp.tile([128, 3, 512], f32, tag="lp", name="lpw")
                nc.tensor.matmul(wp[:, 0, :], wsrc[:, :128], wsrc[:],
                                 start=True, stop=True)

            # den tree scratch (bf16 pairwise tree over the resident P tile)
            acc1 = accp.tile([128, 18, 512], bf, name="acc1")
            acc2 = accp.tile([128, 9, 512], bf, name="acc2")
            acc3 = accp.tile([128, 4, 512], bf, name="acc3")
            acc4 = accp.tile([128, 2, 512], bf, name="acc4")
            acc5 = accp.tile([128, 512], bf, name="acc5")
            acc6 = accp.tile([128, 512], bf, name="acc6")
            den = accp.tile([128, 512], bf, name="den")
            sden = accp.tile([128, 512], f32, name="sden")
            sinv = accp.tile([128, 512], f32, name="sinv")
            # staging for the PV accumulator so its PSUM bank frees long
            # before the den chain finishes (double-buffered)
            o3us = [accp.tile([128, 512], f32, name=f"o3u{i}")
                    for i in range(2)]

            # double-buffered resident P tiles; the tail chunk's garbage
            # rows [72:128] are zeroed once so the den tree reads zeros
            # (the PV matmul is safe regardless: V rows 72: are zero)
            pts = [consts.tile([128, NKC, 512], bf, name=f"pt{i}")
                   for i in range(2)]
            # (partition base must be 32-aligned: clear the whole slot;
            # exp overwrites rows [:72] every iteration)
            nc.vector.memset(pts[0][:, NKC - 1, :], 0.0)
            nc.vector.memset(pts[1][:, NKC - 1, :], 0.0)

            # software pipeline: each entry is (pv_closure, finish_closure);
            # the last two PV groups of every head are deliberately held
            # back and drained at the START of the next head, filling the
            # pipeline-fill bubble while that head's first exp is in flight
            pending_pv = []

            def pop_pv():
                pv, fin = pending_pv.pop(0)
                pv()
                if fin is not None:
                    fin()

            # all K/V loads upfront (resident for both query blocks); the
            # triggers sit ahead of every partition_all_reduce in the
            # gpsimd FIFO so DMAs never wait on compute
            kts, vts = {}, {}
            for h in range(HPC):
                kts[h] = kvp.tile([128, CACHE], bf, tag="kt",
                                  name=f"ktile{h}")
                vts[h] = kvp.tile([128, NKC, 128], bf, tag="vt",
                                  name=f"vtile{h}")
            nc.sync.dma_start(kts[0][:, :256], kt_d.ap()[0][:, :256])
            nc.gpsimd.dma_start(vts[0][:, :6, :], vt_d.ap()[0][:, :6, :])
            nc.sync.dma_start(kts[0][:, 256:1536], kt_d.ap()[0][:, 256:1536])
            nc.sync.dma_start(kts[0][:, 1536:], kt_d.ap()[0][:, 1536:])
            nc.gpsimd.dma_start(vts[0][:, 6:, :], vt_d.ap()[0][:, 6:, :])
            for h in range(1, HPC):
                nc.sync.dma_start(kts[h][:], kt_d.ap()[h])
                nc.gpsimd.dma_start(vts[h][:], vt_d.ap()[h])

            def out_round(r0, r1, nf):
                rs = r1 - r0
                po = popp.tile([128, 512], f32, tag="po", name="po")
                for h in range(HPC):
                    nc.tensor.matmul(
                        po[:rs, :],
                        o3[:, h, r0:r1],
                        w2[:, h, nf, :],
                        start=(h == 0), stop=(h == HPC - 1))
                outf = outsp.tile([128, 512], bf, tag="of", name="of")
                nc.vector.tensor_copy(outf[:rs, :], po[:rs, :])
                deng = (nc.sync, nc.scalar, nc.gpsimd)[nf]
                deng.dma_start(
                    out_d.ap()[r0:r1, nf * 512:(nf + 1) * 512],
                    outf[:rs, :])

            # qb0's out-proj rounds, interleaved into qb1's stream
            deferred = [(r0, r1, nf)
                        for (r0, r1) in ((0, 128), (128, 256),
                                         (256, 384), (384, 512))
                        for nf in range(3)]

            TT = nc.vector.tensor_tensor

            def tree_levels(qw):
                TT(acc2[:, :, :qw], acc1[:, 0:9, :qw],
                   acc1[:, 9:18, :qw], AluOpType.add)
                TT(acc3[:, :, :qw], acc2[:, 0:4, :qw],
                   acc2[:, 4:8, :qw], AluOpType.add)
                TT(acc4[:, :, :qw], acc3[:, 0:2, :qw],
                   acc3[:, 2:4, :qw], AluOpType.add)
                TT(acc5[:, :qw], acc4[:, 0, :qw], acc4[:, 1, :qw],
                   AluOpType.add)
                TT(acc6[:, :qw], acc5[:, :qw], acc2[:, 8, :qw],
                   AluOpType.add)

            for qb in range(2):
                qw = QB[qb]
                q0 = 512 * qb
                for h in range(HPC):
                    kt, vt = kts[h], vts[h]
                    last = (qb == 1 and h == HPC - 1)
                    o3u = o3us[(qb * HPC + h) % 2]
                    opsum = opsp.tile([128, 512], f32, tag="opsum",
                                      name="opsum")
                    pt = pts[(qb * HPC + h) % 2]
                    for g in range(NGR):
                        jj = 3 * g
                        nch = min(3, NKC - jj)
                        pw = 128 if nch == 3 else TAIL  # exp partition rows
                        lp = lpp.tile([128, 3, 512], f32, tag="lp", name="lp")
                        for u in range(nch):
                            j = jj + u
                            kw = TAIL if j == NKC - 1 else 128
                            nc.tensor.matmul(
                                lp[:kw, u, :qw],
                                kt[:, j * 128:j * 128 + kw],
                                qt[:, h, q0:q0 + qw],
                                start=True, stop=True)
                        nc.scalar.activation(
                            out=pt[:pw, jj:jj + nch, :qw],
                            in_=lp[:pw, :nch, :qw], func=AF.Exp)

                        def pv_group(jj=jj, nch=nch, pt=pt, vt=vt,
                                     opsum=opsum, qw=qw):
                            for u in range(nch):
                                j = jj + u
                                nc.tensor.matmul(
                                    opsum[:, :qw],
                                    vt[:, j, :],
                                    pt[:, j, :qw],
                                    start=(j == 0), stop=(j == NKC - 1))

                        if g < NGR - 1:
                            fin = None
                        else:
                            def fin(h=h, q0=q0, qw=qw, o3u=o3u, opsum=opsum):
                                # free the PV PSUM bank right away, then
                                # normalize; the reduce ran during g12
                                nc.vector.tensor_copy(o3u[:, :qw],
                                                      opsum[:, :qw])
                                nc.vector.reciprocal_approx_fast(
                                    out=sinv[:, :qw], in_=sden[:, :qw])
                                nc.vector.tensor_tensor(
                                    o3[:, h, q0:q0 + qw], o3u[:, :qw],
                                    sinv[:, :qw], AluOpType.mult)
                        pending_pv.append((pv_group, fin))
                        if g < NGR - 2:
                            while len(pending_pv) > 1:
                                pop_pv()

                        # den tree emitted piecewise so only the tiny final
                        # add + partition-reduce trail the last exp
                        if 6 <= g <= 11:
                            u0 = 3 * (g - 6)
                            TT(acc1[:, u0:u0 + 3, :qw],
                               pt[:, u0:u0 + 3, :qw],
                               pt[:, u0 + 18:u0 + 21, :qw],
                               AluOpType.add)
                            if g == 11:
                                tree_levels(qw)
                        elif g == NGR - 1:
                            TT(den[:, :qw], acc6[:, :qw],
                               pt[:, 36, :qw], AluOpType.add)
                            nc.gpsimd.partition_all_reduce(
                                sden[:, :qw], den[:, :qw], 128,
                                bass_isa.ReduceOp.add)
                        if qb == 1 and deferred and (
                                (h == 1 and g >= 6) or (h == 2 and g < 6)):
                            out_round(*deferred.pop(0))

            while pending_pv:
                pop_pv()
            # drain any undeferred qb0 rounds, then qb1's tail rounds
            # (double-buffered on the now-idle QK PSUM banks)
            tail_rounds = list(deferred)
            deferred.clear()
            tail_rounds += [(r0, r1, nf)
                            for (r0, r1) in ((512, 640), (640, 768),
                                             (768, 780))
                            for nf in range(3)]
            for (r0, r1, nf) in tail_rounds:
                rs = r1 - r0
                po = lpp.tile([128, 3, 512], f32, tag="lp", name="lp")
                for h in range(HPC):
                    nc.tensor.matmul(
                        po[:rs, 0, :],
                        o3[:, h, r0:r1],
                        w2[:, h, nf, :],
                        start=(h == 0), stop=(h == HPC - 1))
                outf = outsp.tile([128, 512], bf, tag="of", name="of")
                nc.vector.tensor_copy(outf[:rs, :], po[:rs, 0, :])
                deng = (nc.sync, nc.scalar, nc.gpsimd)[nf]
                deng.dma_start(
                    out_d.ap()[r0:r1, nf * 512:(nf + 1) * 512],
                    outf[:rs, :])

    nc.finalize()
    return nc


# ---------------------------------------------------------------------------
def _cache_plan(current_start, global_end_index, local_end_index, s, kv_size,
                frame_seqlen):
    """Numpy re-implementation of the reference's cache roll/update/window
    logic, tracking only *indices*: returns (old_cache_rows, new_rows) such
    that the attended key set == cache[old_cache_rows] ++ new[new_rows]."""
    current_end = current_start + s
    sink_tokens = SINK_SIZE * frame_seqlen

    # each cache slot: kind 0 -> original cache row idx, kind 1 -> new row idx
    kind = np.zeros(kv_size, dtype=np.int64)
    idx = np.arange(kv_size, dtype=np.int64)

    if (LOCAL_ATTN_SIZE != -1 and current_end > global_end_index
            and s + local_end_index > kv_size):
        num_evicted = s + local_end_index - kv_size
        num_rolled = local_end_index - num_evicted - sink_tokens
        src0 = sink_tokens + num_evicted
        kind[sink_tokens:sink_tokens + num_rolled] = \
            kind[src0:src0 + num_rolled]
        idx[sink_tokens:sink_tokens + num_rolled] = \
            idx[src0:src0 + num_rolled]
        new_local_end = (local_end_index + current_end - global_end_index
                         - num_evicted)
    else:
        new_local_end = local_end_index + current_end - global_end_index
    local_start = new_local_end - s
    is_recompute = (current_end <= global_end_index) and (current_start > 0)
    write_start = max(local_start, sink_tokens) if is_recompute \
        else local_start
    off = max(0, write_start - local_start)
    wl = max(0, new_local_end - write_start)
    if wl > 0:
        kind[write_start:new_local_end] = 1
        idx[write_start:new_local_end] = off + np.arange(wl)

    if sink_tokens > 0:
        budget = MAX_ATTN - sink_tokens
        if budget > 0:
            lo = max(sink_tokens, new_local_end - budget)
            sel = np.concatenate([np.arange(sink_tokens),
                                  np.arange(lo, new_local_end)])
        else:
            sel = np.arange(sink_tokens)
    else:
        ws = max(0, new_local_end - MAX_ATTN)
        sel = np.arange(ws, new_local_end)

    k_kind, k_idx = kind[sel], idx[sel]
    old_rows = k_idx[k_kind == 0]
    new_rows = k_idx[k_kind == 1]
    return old_rows, new_rows


def _rope_tables_head(freqs_real, freqs_imag, f, h, w, start_frame,
                      gq, gk):
    """Per-head (S,128) cos table and sign-folded sin table, with the
    per-head g block folded in (g must be identical across heads)."""
    c = HD // 2  # 64
    c0 = c - 2 * (c // 3)
    c1 = c // 3
    fr = np.asarray(freqs_real, np.float32)
    fi = np.asarray(freqs_imag, np.float32)
    s = f * h * w
    assert s == S
    fidx = np.arange(s) // (h * w)
    hidx = (np.arange(s) // w) % h
    widx = np.arange(s) % w
    fr_pos = np.concatenate([
        fr[start_frame + fidx][:, :c0],
        fr[hidx][:, c0:c0 + c1],
        fr[widx][:, c0 + c1:c0 + 2 * c1],
    ], axis=1)  # (S, 64)
    fi_pos = np.concatenate([
        fi[start_frame + fidx][:, :c0],
        fi[hidx][:, c0:c0 + c1],
        fi[widx][:, c0 + c1:c0 + 2 * c1],
    ], axis=1)
    C1 = np.repeat(fr_pos, 2, axis=1)              # (S, 128)
    Sg = np.empty((s, HD), np.float32)
    Sg[:, 0::2] = -fi_pos                          # y_even = xe*c - xo*si
    Sg[:, 1::2] = fi_pos                           # y_odd  = xo*c + xe*si
    gq = np.asarray(gq, np.float32).reshape(NH, HD)
    gk = np.asarray(gk, np.float32).reshape(NH, HD)
    assert np.array_equal(gq, np.broadcast_to(gq[0], gq.shape)) and \
        np.array_equal(gk, np.broadcast_to(gk[0], gk.shape)), \
        "g must be identical across heads for shared rope tables"
    gqb, gkb = gq[0], gk[0]
    gqs = gqb.reshape(-1, 2)[:, ::-1].reshape(-1)
    gks = gkb.reshape(-1, 2)[:, ::-1].reshape(-1)
    return (C1 * gqb[None, :], Sg * gqs[None, :],
            C1 * gkb[None, :], Sg * gks[None, :])


# ---------------------------------------------------------------------------
def kernel(x, cache_k, cache_v, freqs_real, freqs_imag,
           wq, bq, wk, bk, wv, bv, wo, bo, gq, gk,
           f_frames, height, width, current_start, global_end_index,
           local_end_index):
    global LAST_RUNS
    LAST_RUNS = []

    x = np.asarray(x, np.float32)
    cache_k = np.asarray(cache_k, np.float32)
    cache_v = np.asarray(cache_v, np.float32)
    wq = np.asarray(wq, np.float32)
    wk = np.asarray(wk, np.float32)
    wv = np.asarray(wv, np.float32)
    wo = np.asarray(wo, np.float32)
    bo = np.asarray(bo, np.float32)
    f = int(f_frames)
    h = int(height)
    w = int(width)
    current_start = int(current_start)
    global_end_index = int(global_end_index)
    local_end_index = int(local_end_index)

    assert x.shape == (1, S, DIM)
    for b in (bq, bk, bv):
        assert not np.any(np.asarray(b)), "nonzero qkv bias unsupported"

    frame_seqlen = h * w
    start_frame = current_start // frame_seqlen

    # ---- launch 1: projections + RMS + RoPE (pair-split, transposed) ----
    Cq, Sq, Ck, Sk = _rope_tables_head(freqs_real, freqs_imag, f, h, w,
                                       start_frame, gq, gk)
    att_sc = 1.0 / float(np.sqrt(HD))   # q-only scale, folded into tables

    def _wtiles(W):
        return np.ascontiguousarray(
            W.reshape(12, 128, NMCH, 128).transpose(2, 1, 0, 3)).astype(BF16)

    wtA = _wtiles(np.concatenate([wq, wv[:, :768]], axis=1))
    wtB = _wtiles(np.concatenate([wk, wv[:, 768:]], axis=1))
    xT = x[0].T.astype(BF16)                                # (1536, 1560)
    permM = np.zeros((128, 128), BF16)
    idx128 = np.arange(128)
    permM[idx128 ^ 1, idx128] = 1.0

    ctA = np.ascontiguousarray((Cq * att_sc).T).astype(BF16)  # (128, S)
    stA = np.ascontiguousarray((Sq * att_sc).T).astype(BF16)
    ctB = np.ascontiguousarray(Ck.T).astype(BF16)
    stB = np.ascontiguousarray(Sk.T).astype(BF16)

    nc1 = _CACHED.get("l1")
    if nc1 is None:
        nc1 = _CACHED["l1"] = _build_launch1()

    in_maps1 = []
    for c in range(NCORES):
        p = c // 2
        r0, r1 = p * RPB, (p + 1) * RPB
        xt_c = np.ascontiguousarray(
            xT[:, r0:r1].reshape(12, 128, RPB).transpose(1, 0, 2))
        if c % 2 == 0:
            in_maps1.append({"xt": xt_c, "wt": wtA, "perm": permM,
                             "ct": ctA[:, r0:r1].copy(),
                             "st": stA[:, r0:r1].copy()})
        else:
            in_maps1.append({"xt": xt_c, "wt": wtB, "perm": permM,
                             "ct": ctB[:, r0:r1].copy(),
                             "st": stB[:, r0:r1].copy()})
    res1 = bass_utils.run_bass_kernel_spmd(nc1, in_maps1,
                                           core_ids=list(range(NCORES)))
    LAST_RUNS.append(res1)
    # transposed outputs: [18, 128, 390] per core -> head-major T tensors
    QT = np.concatenate(
        [res1.results[2 * p]["out"][:12] for p in range(4)], axis=2)
    KnewT = np.concatenate(
        [res1.results[2 * p + 1]["out"][:12] for p in range(4)], axis=2)
    VnewT = np.concatenate(
        [np.concatenate([res1.results[2 * p]["out"][12:],
                         res1.results[2 * p + 1]["out"][12:]], axis=0)
         for p in range(4)], axis=2)          # (12, 128, 1560)

    # ---- host glue: effective K/V assembly ----
    old_rows, new_rows = _cache_plan(current_start, global_end_index,
                                     local_end_index, S, cache_k.shape[1],
                                     frame_seqlen)
    n_keys = len(old_rows) + len(new_rows)
    assert n_keys == CACHE, f"unexpected key count {n_keys}"

    n_old = len(old_rows)
    ktT = np.empty((NH, HD, CACHE), BF16)
    ktT[:, :, :n_old] = cache_k[0, old_rows].astype(BF16).transpose(1, 2, 0)
    ktT[:, :, n_old:] = KnewT[:, :, new_rows]
    kt = ktT
    V_pad = np.zeros((NKC * 128, NH, HD), BF16)
    V_pad[:n_old] = cache_v[0, old_rows].astype(BF16)
    V_pad[n_old:CACHE] = VnewT[:, :, new_rows].transpose(2, 0, 1)
    vt = np.ascontiguousarray(
        V_pad.reshape(NKC, 128, NH, HD).transpose(2, 1, 0, 3))
    w2 = np.ascontiguousarray(
        wo.reshape(12, 128, 3, 512).transpose(1, 0, 2, 3)).astype(BF16)

    nc2 = _CACHED.get("l2")
    if nc2 is None:
        nc2 = _CACHED["l2"] = _build_launch2()

    in_maps2 = []
    for c in range(NCORES):
        hg, qh = c // 2, c % 2
        h0 = hg * HPC
        r0, r1 = qh * QPC, (qh + 1) * QPC
        qt_c = np.zeros((128, HPC, 784), BF16)
        qt_c[:, :, :QPC] = QT[h0:h0 + HPC, :, r0:r1].transpose(1, 0, 2)
        in_maps2.append({
            "qt": qt_c,
            "kt": np.ascontiguousarray(kt[h0:h0 + HPC]),
            "vt": np.ascontiguousarray(vt[h0:h0 + HPC]),
            "w2": np.ascontiguousarray(w2[:, h0:h0 + HPC]),
        })
    res2 = bass_utils.run_bass_kernel_spmd(nc2, in_maps2,
                                           core_ids=list(range(NCORES)))
    LAST_RUNS.append(res2)

    out = np.zeros((S, DIM), np.float32)
    for c in range(NCORES):
        hg, qh = c // 2, c % 2
        out[qh * QPC:(qh + 1) * QPC] += \
            res2.results[c]["outp"].astype(np.float32)
    out += bo.reshape(1, DIM)
    return out.reshape(1, S, DIM)

